# revision 2
# baseline (speedup 1.0000x reference)
"""Trainium2 Bass kernel for nn_NewTable (histogram_binning / 35-entry GELU table).

The reference op is an elementwise fp16 piecewise-linear GELU table:
  - core region [-4, 4): 32 PL segments sampling exact erf-GELU at
    quarter-binade knots (max |PL - gelu| ~ 1.6e-3, i.e. ~1.2e-4 of absmax),
  - tail x >= 4: y = fp16(4 + fp16(0.99951171875 * fp16(x - 4)))
    (ms9 == 2**-16 exactly, 65504 * 2**-16 == 0.99951171875),
  - tail x <= -4: y == fp16 constant ~ -1.2666e-4 (gelu(x) there is ~-0,
    abs diff ~1.3e-4, i.e. ~1e-5 of absmax).

Kernel computes  y = min(gelu_ACT(x), 4 + 0.99951171875 * relu(x - 4))
with the tail chain rounded fp16-exactly (matches the reference bit-for-bit
on the whole tail; verified over every fp16 value in [4, 16]).

Sharding: pure data parallel over the leading dim (8 batches -> 8 cores).
Per core: [2048, 4096] fp16 in + out = 32 MiB of HBM traffic ~ memory
roofline ~90 us; ACT gelu (~59 us), DVE (~55 us) and GPSIMD (~56 us) all
hide under the DMA shadow.
"""

import os
import sys

import numpy as np

for _p in ("/opt/trn_rl_repo", "/root/.axon_site/_ro/trn_rl_repo"):
    if os.path.isdir(_p) and _p not in sys.path:
        sys.path.append(_p)

N_CORES = 8
ROWS, COLS = 2048, 4096  # per-core shard of x: x[c] in [8, 2048, 4096]
P = 128
NTILES = ROWS // P  # 16 tiles of [128, 4096] fp16 (1 MiB each)
C_TAIL = 0.99951171875  # 65504 * 2**-16 == fp32(fp16(1.0)/fp16(65500.0)) * 65504

_CACHE = {}


def _build_nc():
    import concourse.bacc as bacc
    import concourse.tile as tile
    from concourse import mybir

    nc = bacc.Bacc(
        "TRN2",
        target_bir_lowering=False,
        debug=False,
        num_devices=N_CORES,
    )
    f16 = mybir.dt.float16
    x = nc.dram_tensor("x", [ROWS, COLS], f16, kind="ExternalInput").ap()
    y = nc.dram_tensor("y", [ROWS, COLS], f16, kind="ExternalOutput").ap()
    xt = x.rearrange("(n p) m -> n p m", p=P)
    yt = y.rearrange("(n p) m -> n p m", p=P)

    from contextlib import ExitStack

    with tile.TileContext(nc) as tc, ExitStack() as ctx:
        in_pool = ctx.enter_context(tc.tile_pool(name="in", bufs=3))
        g_pool = ctx.enter_context(tc.tile_pool(name="g", bufs=2))
        r_pool = ctx.enter_context(tc.tile_pool(name="r", bufs=2))
        t_pool = ctx.enter_context(tc.tile_pool(name="t", bufs=2))
        out_pool = ctx.enter_context(tc.tile_pool(name="out", bufs=3))

        for i in range(NTILES):
            tx = in_pool.tile([P, COLS], f16)
            nc.sync.dma_start(tx[:], xt[i, :, :])

            # ACT: g = gelu(x)   (erf-based hardware gelu, fp32 internal)
            g = g_pool.tile([P, COLS], f16)
            nc.scalar.activation(g[:], tx[:], mybir.ActivationFunctionType.Gelu)

            # GPSIMD: r = fp16(max(x - 4, 0))   (exact for x <= 16)
            r = r_pool.tile([P, COLS], f16)
            nc.gpsimd.tensor_scalar(
                r[:], tx[:], 4.0, 0.0, mybir.AluOpType.subtract, mybir.AluOpType.max
            )
            # DVE: inner = fp16(C_TAIL * r);  T = fp16(inner + 4)
            inner = r_pool.tile([P, COLS], f16, tag="inner")
            nc.vector.tensor_scalar(
                inner[:], r[:], C_TAIL, None, mybir.AluOpType.mult
            )
            T = t_pool.tile([P, COLS], f16)
            nc.vector.tensor_scalar(T[:], inner[:], 4.0, None, mybir.AluOpType.add)

            # DVE: y = min(g, T)
            out = out_pool.tile([P, COLS], f16)
            nc.vector.tensor_tensor(out[:], g[:], T[:], mybir.AluOpType.min)

            nc.sync.dma_start(yt[i, :, :], out[:])

    nc.compile()
    return nc


def _get_nc():
    if "nc" not in _CACHE:
        _CACHE["nc"] = _build_nc()
    return _CACHE["nc"]


def run_on_hw(x_np, trace=False, **trace_kwargs):
    """x_np: [8, 2048, 4096] fp16 -> (y [8,2048,4096] fp16, BassKernelResults)."""
    from concourse.bass_utils import run_bass_kernel_spmd

    nc = _get_nc()
    in_maps = [
        {"x": np.ascontiguousarray(x_np[c].reshape(ROWS, COLS))}
        for c in range(N_CORES)
    ]
    res = run_bass_kernel_spmd(
        nc, in_maps, list(range(N_CORES)), trace=trace, **trace_kwargs
    )
    y = np.stack([np.asarray(r["y"]).reshape(ROWS, COLS) for r in res.results])
    return y.astype(np.float16), res


def kernel(x, cut_points=None, table=None, mul_scale=None):
    x_np = np.asarray(x)
    assert x_np.shape == (N_CORES, ROWS, COLS), x_np.shape
    x_np = x_np.astype(np.float16, copy=False)
    y, _ = run_on_hw(x_np)
    return y.reshape(N_CORES, ROWS, COLS)


# revision 4
# speedup vs baseline: 238.8723x; 238.8723x over previous
"""Trainium2 Bass kernel for nn_NewTable (histogram_binning / 35-entry GELU table).

The reference op is an elementwise fp16 piecewise-linear GELU table:
  - core region [-4, 4): 32 PL segments sampling exact erf-GELU at
    quarter-binade knots (max |PL - gelu| ~ 1.6e-3, i.e. ~1.2e-4 of absmax),
  - tail x >= 4: y = fp16(4 + fp16(0.99951171875 * fp16(x - 4)))
    (ms9 == 2**-16 exactly, 65504 * 2**-16 == 0.99951171875),
  - tail x <= -4: y == fp16 constant ~ -1.2666e-4 (gelu(x) there is ~-0,
    abs diff ~1.3e-4, i.e. ~1e-5 of absmax).

Kernel computes  y = min(gelu_ACT(x), 4 + 0.99951171875 * relu(x - 4))
with the tail chain rounded fp16-exactly (matches the reference bit-for-bit
on the whole tail; verified over every fp16 value in [4, 16]).

Sharding: pure data parallel over the leading dim (8 batches -> 8 cores).
Per core: [2048, 4096] fp16 in + out = 32 MiB of HBM traffic ~ memory
roofline ~90 us; ACT gelu (~59 us), DVE (~55 us) and GPSIMD (~56 us) all
hide under the DMA shadow.
"""

import os
import sys

import numpy as np

for _p in ("/opt/trn_rl_repo", "/root/.axon_site/_ro/trn_rl_repo"):
    if os.path.isdir(_p) and _p not in sys.path:
        sys.path.append(_p)

N_CORES = 8
ROWS, COLS = 2048, 4096  # per-core shard of x: x[c] in [8, 2048, 4096]
P = 128
NTILES = ROWS // P  # 16 tiles of [128, 4096] fp16 (1 MiB each)
C_TAIL = 0.99951171875  # 65504 * 2**-16 == fp32(fp16(1.0)/fp16(65500.0)) * 65504

_CACHE = {}


def _build_nc():
    import concourse.bacc as bacc
    import concourse.tile as tile
    from concourse import mybir

    nc = bacc.Bacc(
        "TRN2",
        target_bir_lowering=False,
        debug=False,
        num_devices=N_CORES,
    )
    f16 = mybir.dt.float16
    x = nc.dram_tensor("x", [ROWS, COLS], f16, kind="ExternalInput").ap()
    y = nc.dram_tensor("y", [ROWS, COLS], f16, kind="ExternalOutput").ap()
    xt = x.rearrange("(n p) m -> n p m", p=P)
    yt = y.rearrange("(n p) m -> n p m", p=P)

    from contextlib import ExitStack

    with tile.TileContext(nc) as tc, ExitStack() as ctx:
        in_pool = ctx.enter_context(tc.tile_pool(name="in", bufs=4))
        g_pool = ctx.enter_context(tc.tile_pool(name="g", bufs=3))
        r_pool = ctx.enter_context(tc.tile_pool(name="r", bufs=3))
        t_pool = ctx.enter_context(tc.tile_pool(name="t", bufs=3))
        out_pool = ctx.enter_context(tc.tile_pool(name="out", bufs=4))

        for i in range(NTILES):
            tx = in_pool.tile([P, COLS], f16)
            nc.sync.dma_start(tx[:], xt[i, :, :])

            # ACT: g = gelu(x)   (erf-based hardware gelu, fp32 internal)
            g = g_pool.tile([P, COLS], f16)
            nc.scalar.activation(g[:], tx[:], mybir.ActivationFunctionType.Gelu)

            # DVE: r = fp16(max(x - 4, 0))   (exact for x <= 16)
            r = r_pool.tile([P, COLS], f16)
            nc.vector.tensor_scalar(
                r[:], tx[:], 4.0, 0.0, mybir.AluOpType.subtract, mybir.AluOpType.max
            )
            # DVE: r = fp16(C_TAIL * r);  T = fp16(r + 4)   (reference's fp16 chain)
            nc.vector.tensor_scalar(r[:], r[:], C_TAIL, None, mybir.AluOpType.mult)
            T = t_pool.tile([P, COLS], f16)
            nc.vector.tensor_scalar(T[:], r[:], 4.0, None, mybir.AluOpType.add)

            # DVE: y = min(g, T)
            out = out_pool.tile([P, COLS], f16)
            nc.vector.tensor_tensor(out[:], g[:], T[:], mybir.AluOpType.min)

            nc.sync.dma_start(yt[i, :, :], out[:])

    nc.compile()
    return nc


def _get_nc():
    if "nc" not in _CACHE:
        _CACHE["nc"] = _build_nc()
    return _CACHE["nc"]


def run_on_hw(x_np, trace=False, **trace_kwargs):
    """x_np: [8, 2048, 4096] fp16 -> (y [8,2048,4096] fp16, BassKernelResults)."""
    from concourse.bass_utils import run_bass_kernel_spmd

    nc = _get_nc()
    in_maps = [
        {"x": np.ascontiguousarray(x_np[c].reshape(ROWS, COLS))}
        for c in range(N_CORES)
    ]
    res = run_bass_kernel_spmd(
        nc, in_maps, list(range(N_CORES)), trace=trace, **trace_kwargs
    )
    y = np.stack([np.asarray(r["y"]).reshape(ROWS, COLS) for r in res.results])
    return y.astype(np.float16), res


def kernel(x, cut_points=None, table=None, mul_scale=None):
    x_np = np.asarray(x)
    assert x_np.shape == (N_CORES, ROWS, COLS), x_np.shape
    x_np = x_np.astype(np.float16, copy=False)
    y, _ = run_on_hw(x_np)
    return y.reshape(N_CORES, ROWS, COLS)
